# revision 4
# baseline (speedup 1.0000x reference)
"""AugmentedLstm Trainium2 kernel — 8 NeuronCores, self-contained.

B=32, T=1024, D=768, H=768.
  proj = inputs @ W_in.T + b_in                    [B,T,6H]
  recurrence over T:  ps = h @ W_s.T + b_s         [B,5H]
    i,f,g,o = sig/sig/tanh/sig(pi+ps); c = i*g + f*c; out0 = o*tanh(c)
    hw = sig(pi4+ps4); out = hw*out0 + (1-hw)*pi5 ; y = out*mask
  (h/c freezing past sequence length never affects the masked y output.)

Distribution: tensor-parallel over the hidden dim (TP-6).
  - cores 0..5 each own one 128-wide H-shard (of each gate block);
    cores 6,7 run the same program on zeroed weights (outputs ignored).
  - Phase 0 (x all-gather): the host uploads only a 4-batch shard of x to
    each core ([4,T,D] bf16, so the global sharded array is just x itself);
    the cores then rebuild the full x in internal DRAM by broadcasting
    [128-token, D] SBUF tiles to all 8 peers with remote_dma_broadcast
    (2-slot rotation, receiver drains to DRAM, ACK via
    remote_sem_update_broadcast). This cuts host->device upload ~8x vs
    uploading the replicated x — the axon tunnel (~40 MB/s aggregate) is
    the end-to-end bottleneck, not the device.
  - Phase 1 (input projection, column-split): each core streams all tokens,
    transposes input tiles on the PE (via identity matmul), and computes its
    pi.T slice -> internal DRAM "pi" [128, t, chunk(7), b]; chunks 0-4 gate
    pre-activations, 5 highway bypass, 6 = sequence mask (broadcast across
    partitions with a rank-1 ones x maskrow matmul).
  - Phase 2 (recurrence): all state transposed [H-shard=128, B=32]. Per step
    30 matmuls (bf16 W stationary, arrived h moving), fp32 gates on DVE/ACT,
    h_next cast to bf16 and pushed to all 8 cores' SBUF with
    remote_dma_broadcast into slot = own partition id; 4-deep recv rotation
    (the h data dependency itself provides cross-core flow control).
    y is stored per step in bf16 to internal DRAM [128, T, 32].
  - Phase 3 (static post-pass): y read back [128,128]-tilewise, DVE 32x32
    block-transposed (block swap folded into the store APs), int8-quantized
    with a per-(4t, b, 32h)-tile f32 scale, stored as y[T, B, 128] + scales.
    This halves the download again (rel-err 8.5e-3 vs the 2e-2 budget).
  - Host: the shard_map'd executable is jit-cached; donated output buffers
    are created on device (no zero upload); only cores 0-5's y/scale shards
    are downloaded + dequantized in threads.

  End-to-end wall is dominated by the ~40 MB/s axon tunnel: ~65 MiB up +
  ~25 MiB down ≈ 2.3s + ~0.1s device exec + ~0.4s host/dispatch.
"""

import sys

for _p in ("/opt/trn_rl_repo", "/opt/pypackages"):
    if _p not in sys.path:
        sys.path.insert(0, _p)

import numpy as np
import ml_dtypes

import concourse.bass as bass
import concourse.mybir as mybir
from concourse import bacc
from concourse.bass_utils import run_bass_kernel_spmd

F32 = mybir.dt.float32
BF16 = mybir.dt.bfloat16
AF = mybir.ActivationFunctionType

B, D, H = 32, 768, 768
NCORES = 8
TPD = 6      # active tensor-parallel cores
HC = 128     # H-shard width per core
NG = 5       # recurrent gate blocks (i,f,g,o,hw)
NPI = 6      # pi blocks per step (5 gates + highway)
NKD = 6      # 128-wide contraction chunks over D=H=768
BSH = B // NCORES   # batch shard per core in phase 0


def build_program(T):
    assert T % 16 == 0
    NTB = T * B // 512          # 512-token blocks in phase 1
    NJ = T // 4                 # phase-2 loop iterations (4 steps each)
    NXT = BSH * T // 128        # phase-0 [128,D] tiles per core

    nc = bacc.Bacc("TRN2", target_bir_lowering=False, debug=False,
                   num_devices=NCORES)

    # ---------------- DRAM ----------------
    xsh = nc.dram_tensor("xsh", [BSH, T, D], BF16, kind="ExternalInput").ap()
    w1t = nc.dram_tensor("w1t", [D, NPI * HC], BF16, kind="ExternalInput").ap()
    w2t = nc.dram_tensor("w2t", [H, NG * HC], BF16, kind="ExternalInput").ap()
    b1d = nc.dram_tensor("b1", [HC, NPI], F32, kind="ExternalInput").ap()
    b2d = nc.dram_tensor("b2", [HC, NG], F32, kind="ExternalInput").ap()
    identd = nc.dram_tensor("ident", [128, 128], BF16, kind="ExternalInput").ap()
    onesd = nc.dram_tensor("ones1", [1, 128], BF16, kind="ExternalInput").ap()
    mrowd = nc.dram_tensor("mrow", [1, T * 32], BF16, kind="ExternalInput").ap()
    xfull = nc.dram_tensor("xfull", [B, T, D], BF16, kind="Internal").ap()
    pi = nc.dram_tensor("pi", [128, T + 8, 7, 32], F32, kind="Internal").ap()
    ydram = nc.dram_tensor("ydram", [128, T, 32], BF16, kind="Internal").ap()
    # phase 3 rewrites y as [t, batch, h-shard], int8-quantized with one f32
    # scale per (4t, b, 32h) tile — halves the (tunnel-bound) download again.
    yout = nc.dram_tensor("y", [T, B, HC], mybir.dt.int8,
                          kind="ExternalOutput").ap()
    yscd = nc.dram_tensor("yscale", [T // 4, 128], F32,
                          kind="ExternalOutput").ap()

    # ---------------- SBUF ----------------
    sb = nc.alloc_sbuf_tensor
    w1_sb = sb("w1_sb", [128, NKD * NPI * HC], BF16)
    w2_sb = sb("w2_sb", [128, NKD * NG * HC], BF16)
    b1_sb = sb("b1_sb", [128, NPI], F32)
    b2_sb = sb("b2_sb", [128, NG], F32)
    id_sb = sb("id_sb", [128, 128], BF16)
    on_sb = sb("on_sb", [1, 128], BF16)
    mr_sb = sb("mr_sb", [1, T * 32], BF16)
    xsend = [sb(f"xsend{m}", [128, D], BF16) for m in range(2)]
    xrecv = [sb(f"xrecv{m}", [128, NCORES * D], BF16) for m in range(2)]
    in_sb = [sb(f"in_sb{u}", [128, D], BF16) for u in range(8)]
    rhs_sb = [sb(f"rhs_sb{c}", [128, 2 * 512], BF16) for c in range(NKD)]
    piout = [sb(f"piout{m}", [128, 512], F32) for m in range(2)]
    mout = [sb(f"mout{m}", [128, 512], F32) for m in range(2)]

    recv = [sb(f"recv{s}", [128, NCORES * 32], BF16) for s in range(4)]
    pib = [sb(f"pib{s}", [128, 7 * 32], F32) for s in range(4)]
    send = [sb(f"send{p}", [128, 32], BF16) for p in range(2)]
    ybuf = [sb(f"ybuf{s}", [128, 32], BF16) for s in range(4)]
    ytin = [sb(f"ytin{u}", [128, 128], BF16) for u in range(4)]
    ytr = [sb(f"ytr{u}", [128, 128], BF16) for u in range(4)]
    q8 = [sb(f"q8_{u}", [128, 128], mybir.dt.int8) for u in range(4)]
    rsc = [sb(f"rsc{u}", [128, 1], F32) for u in range(4)]
    rmax = sb("rmax", [128, 1], F32)
    rinv = sb("rinv", [128, 1], F32)
    ceps = sb("ceps", [128, 1], F32)
    c127 = sb("c127", [128, 1], F32)
    ctile = sb("ctile", [128, 32], F32)
    sg = [sb(f"sg{i}", [128, 32], F32) for i in range(NG)]
    ag = [sb(f"ag{i}", [128, 32], F32) for i in range(NG)]
    tmp0 = sb("tmp0", [128, 32], F32)
    tmp1 = sb("tmp1", [128, 32], F32)
    tanhc = sb("tanhc", [128, 32], F32)
    out0 = sb("out0", [128, 32], F32)
    htile = sb("htile", [128, 32], F32)

    # ---------------- PSUM ----------------
    ptr = [nc.alloc_psum_tensor(f"ptr{p}", [128, 512], BF16) for p in range(2)]
    pmm = [nc.alloc_psum_tensor(f"pmm{p}", [128, 512], F32) for p in range(2)]
    pmsk = nc.alloc_psum_tensor("pmsk", [128, 512], F32)
    p2 = [nc.alloc_psum_tensor(f"p2_{p}", [128, NG * 32], F32) for p in range(2)]

    # ---------------- semaphores ----------------
    sem = nc.alloc_semaphore
    WLD, TRC, MMD, PIA = sem("WLD"), sem("TRC"), sem("MMD"), sem("PIA")
    INS = [sem("INS0"), sem("INS1")]
    PIS = [sem("PIS0"), sem("PIS1")]
    MSS = [sem("MSS0"), sem("MSS1")]
    PTD, MSD, MSC = sem("PTD"), sem("MSD"), sem("MSC")
    RS = [sem(f"RS{s}") for s in range(4)]
    PID = [sem(f"PID{s}") for s in range(4)]
    YS = [sem(f"YS{s}") for s in range(4)]
    YLD, TRD, YSD = sem("YLD"), sem("TRD"), sem("YSD")
    LS = [sem("LS0"), sem("LS1")]
    PR, PSD = sem("PR"), sem("PSD")
    Asem, Bsem, Cd, Dd, Z = (sem("A"), sem("B"), sem("Cd"), sem("Dd"),
                              sem("Z"))
    PF, YB, SD = sem("PF"), sem("YB"), sem("SD")
    XLD, XLS, XLS2, XPR, XCP = (sem("XLD"), sem("XLS"), sem("XLS2"),
                                sem("XPR"), sem("XCP"))
    XRS = [sem("XRS0"), sem("XRS1")]
    XACK = [sem("XACK0"), sem("XACK1")]

    tens, vec, scl, gp, syn = nc.tensor, nc.vector, nc.scalar, nc.gpsimd, nc.sync

    def w1tile(kd, m):
        return w1_sb.ap()[:, kd * (NPI * HC) + m * HC:
                          kd * (NPI * HC) + (m + 1) * HC]

    def w2tile(kd, m):
        return w2_sb.ap()[:, kd * (NG * HC) + m * HC:
                          kd * (NG * HC) + (m + 1) * HC]

    # ============ preamble: constant loads ============
    syn.dma_start(w1_sb.ap().rearrange("p (k c) -> p k c", k=NKD),
                  w1t.rearrange("(k p) c -> p k c", p=128)).then_inc(WLD, 16)
    syn.dma_start(w2_sb.ap().rearrange("p (k c) -> p k c", k=NKD),
                  w2t.rearrange("(k p) c -> p k c", p=128)).then_inc(WLD, 16)
    syn.dma_start(b1_sb.ap(), b1d).then_inc(WLD, 16)
    syn.dma_start(b2_sb.ap(), b2d).then_inc(WLD, 16)
    syn.dma_start(id_sb.ap(), identd).then_inc(WLD, 16)
    syn.dma_start(on_sb.ap(), onesd).then_inc(WLD, 16)
    syn.dma_start(mr_sb.ap(), mrowd).then_inc(WLD, 16)
    tens.wait_ge(WLD, 112)
    vec.wait_ge(WLD, 112)
    scl.wait_ge(WLD, 112)
    vec.memset(ceps.ap(), 1e-30)
    vec.memset(c127.ap(), 1.0 / 127.0)

    # ============ phase 0: all-gather x (batch shards -> xfull) ============
    pid_sv = gp.partition_id()
    rdests = [(0, k) for k in range(NCORES)]
    for j in range(NXT):
        slot = j % 2
        bl, t0 = j // 8, 128 * (j % 8)
        # sender: stage own tile
        if j >= 2:
            syn.wait_ge(XLS, 16 * (j - 1))
        syn.dma_start(xsend[slot].ap(),
                      xsh[bl:bl + 1, t0:t0 + 128, :]).then_inc(XLD, 16)
        # broadcast tile j to slot `slot` of every core
        gp.wait_ge(XLD, 16 * (j + 1))
        if j >= 2:
            gp.wait_ge(XACK[slot], 16 * (j // 2))
        gp.remote_dma_broadcast(
            xrecv[slot].ap()[:, bass.ts(pid_sv, D)], xsend[slot].ap(),
            remote_sem=XRS[slot], local_sem=XLS, rdests=rdests,
        ).then_inc(XPR, 1)
        gp.wait_ge(XPR, 2 * j + 1)
        gp.trigger_dma(1)
        # receiver: drain round j (all 8 senders) to xfull
        syn.wait_ge(XRS[slot], 16 * (j // 2 + 1))
        for s in range(NCORES):
            syn.dma_start(
                xfull[BSH * s + bl:BSH * s + bl + 1, t0:t0 + 128, :],
                xrecv[slot].ap()[:, s * D:(s + 1) * D],
            ).then_inc(XCP, 16)
        # ACK: tell every sender this core drained round j
        gp.wait_ge(XCP, 128 * (j + 1))
        gp.remote_sem_update_broadcast(
            remote_sem=XACK[slot], local_sem=XLS2, rdests=rdests,
        ).then_inc(XPR, 1)
        gp.wait_ge(XPR, 2 * j + 2)
        gp.trigger_dma(1)
    # all local drains done -> xfull complete on this core
    syn.wait_ge(XCP, 128 * NXT)

    # ============ phase 1: input projection (python-unrolled) ============
    for tb in range(NTB):
        half = tb % 2
        # token loads: 4 tiles x [128 = 4t x 32b, 768]
        if tb >= 2:
            syn.wait_ge(PTD, 6 * (tb - 1))
        for u in range(4):
            for v in range(4):
                tq = tb * 16 + 4 * u + v
                syn.dma_start(
                    in_sb[4 * half + u].ap()[32 * v:32 * (v + 1), :],
                    xfull[:, tq:tq + 1, :],
                ).then_inc(INS[half], 16)
        # PE transposes: 6 chunk-groups of 4
        for c in range(NKD):
            g = 6 * tb + c
            if c == 0:
                tens.wait_ge(INS[half], 256 * (tb // 2 + 1))
            if g >= 2:
                tens.wait_ge(TRC, g - 1)
            for u in range(4):
                mm = tens.transpose(
                    ptr[c % 2].ap()[:, 128 * u:128 * (u + 1)],
                    in_sb[4 * half + u].ap()[:, 128 * c:128 * (c + 1)],
                    id_sb.ap(),
                )
                if u == 3:
                    mm.then_inc(PTD, 1)
        # DVE: psum -> bf16 rhs tiles
        for c in range(NKD):
            g = 6 * tb + c
            vec.wait_ge(PTD, g + 1)
            if tb >= 2 and c == 0:
                vec.wait_ge(MMD, 6 * (tb - 1))
            vec.tensor_copy(
                rhs_sb[c].ap()[:, half * 512:(half + 1) * 512],
                ptr[c % 2].ap(),
            ).then_inc(TRC, 1)
        # PE: 6 m-groups x 6 kd matmuls
        for m in range(NPI):
            g2 = 6 * tb + m
            if m == 0:
                tens.wait_ge(TRC, 6 * (tb + 1))
            if g2 >= 2:
                tens.wait_ge(PIA, g2 - 1)
            for kd in range(NKD):
                mm = tens.matmul(
                    pmm[m % 2].ap(),
                    w1tile(kd, m),
                    rhs_sb[kd].ap()[:, half * 512:(half + 1) * 512],
                    start=(kd == 0),
                    stop=(kd == NKD - 1),
                )
                if kd == NKD - 1:
                    mm.then_inc(MMD, 1)
        # DVE: + b_in, fp32 out; sync: store to pi
        for m in range(NPI):
            g2 = 6 * tb + m
            vec.wait_ge(MMD, g2 + 1)
            if g2 >= 2:
                vec.wait_ge(PIS[g2 % 2], 16 * (g2 // 2))
            vec.tensor_scalar_add(
                piout[m % 2].ap(), pmm[m % 2].ap(), b1_sb.ap()[:, m:m + 1]
            ).then_inc(PIA, 1)
            syn.wait_ge(PIA, g2 + 1)
            syn.dma_start(
                pi[:, tb * 16:(tb + 1) * 16, m:m + 1, :], piout[m % 2].ap()
            ).then_inc(PIS[g2 % 2], 16)
        # mask broadcast for this block: ones[1,128] x mrow[1,512]
        tens.wait_ge(MSC, tb)
        tens.matmul(
            pmsk.ap(), on_sb.ap(),
            mr_sb.ap()[0:1, tb * 512:(tb + 1) * 512],
            start=True, stop=True,
        ).then_inc(MSD, 1)
        vec.wait_ge(MSD, tb + 1)
        if tb >= 2:
            vec.wait_ge(MSS[half], 16 * (tb // 2))
        vec.tensor_copy(mout[half].ap(), pmsk.ap()).then_inc(MSC, 1)
        syn.wait_ge(MSC, tb + 1)
        syn.dma_start(
            pi[:, tb * 16:(tb + 1) * 16, 6:7, :], mout[half].ap()
        ).then_inc(MSS[half], 16)

    for p_ in range(2):
        syn.wait_ge(PIS[p_], 16 * (NPI * NTB // 2))
        syn.wait_ge(MSS[p_], 16 * (NTB // 2))
    # zero-fill the 8 tail rows of pi (read by harmless tail prefetches)
    TZ = sem("TZ")
    for p_ in range(2):
        vec.wait_ge(PIS[p_], 16 * (NPI * NTB // 2))
    vec.drain()
    vec.memset(piout[0].ap()[:, 0:224], 0.0).then_inc(TZ, 1)
    syn.wait_ge(TZ, 1)
    for r_ in range(8):
        syn.dma_start(pi[:, T + r_:T + r_ + 1, :, :],
                      piout[0].ap()[:, 0:224]).then_inc(TZ, 16)
    syn.wait_ge(TZ, 129)
    nc.all_engine_barrier()

    # ============ phase 2: recurrence ============
    # preamble: zero h broadcast into recv[0], zero c, prefetch pi 0..3
    vec.memset(send[1].ap(), 0.0).then_inc(Z, 1)
    vec.memset(ctile.ap(), 0.0)
    vec.sem_inc(PF, 2)
    gp.wait_ge(Z, 1)
    gp.remote_dma_broadcast(
        recv[0].ap()[:, bass.ts(pid_sv, 32)], send[1].ap(),
        remote_sem=RS[0], local_sem=LS[1], rdests=rdests,
    ).then_inc(PR, 1)
    gp.wait_ge(PR, 1)
    gp.trigger_dma(1)
    for s in range(4):
        syn.dma_start(pib[s].ap(), pi[:, s:s + 1, :, :]).then_inc(PID[s], 16)

    with nc.Fori(0, NJ) as j:
        for s in range(4):
            par = s % 2
            # ---- PE: 5 m-tiles x 6 chunks ----
            tens.wait_ge(PF, j * 4 + (s + 1))
            tens.wait_ge(RS[s], j * 16 + 16)
            for m in range(NG):
                for kd in range(NKD):
                    mm = tens.matmul(
                        p2[par].ap()[:, 32 * m:32 * (m + 1)],
                        w2tile(kd, m),
                        recv[s].ap()[:, 32 * kd:32 * (kd + 1)],
                        start=(kd == 0),
                        stop=(kd == NKD - 1),
                    )
                    if kd == NKD - 1:
                        mm.then_inc(PSD, 1)
            # ---- DVE: gate pre-activations ----
            vec.wait_ge(PSD, j * 20 + (5 * s + 5))
            vec.wait_ge(PID[s], j * 16 + 16)
            if True:
                vec.wait_ge(YS[s], j * 16)
                vec.wait_ge(LS[par], j * 32 + (8 * s + (8 if par else 0)))
            for i in range(NG):
                vec.tensor_add(
                    sg[i].ap(), p2[par].ap()[:, 32 * i:32 * (i + 1)],
                    pib[s].ap()[:, 32 * i:32 * (i + 1)],
                ).then_inc(Asem, 1)
            vec.drain().then_inc(PF, 1)
            # ---- ACT: activations with b_s bias ----
            for i in range(NG):
                scl.wait_ge(Asem, j * 20 + (5 * s + i + 1))
                scl.activation(
                    ag[i].ap(), sg[i].ap(),
                    AF.Tanh if i == 2 else AF.Sigmoid,
                    bias=b2_sb.ap()[:, i:i + 1],
                ).then_inc(Bsem, 1)
            # ---- DVE: c update ----
            vec.wait_ge(Bsem, j * 20 + (5 * s + 3))
            vec.tensor_mul(tmp0.ap(), ag[0].ap(), ag[2].ap())
            vec.tensor_mul(tmp1.ap(), ag[1].ap(), ctile.ap())
            vec.drain()
            vec.tensor_add(ctile.ap(), tmp0.ap(), tmp1.ap()).then_inc(Cd, 1)
            scl.wait_ge(Cd, j * 4 + (s + 1))
            scl.activation(tanhc.ap(), ctile.ap(), AF.Tanh).then_inc(Dd, 1)
            # ---- DVE: output, highway, mask, cast ----
            vec.wait_ge(Bsem, j * 20 + (5 * s + 5))
            vec.wait_ge(Dd, j * 4 + (s + 1))
            vec.tensor_mul(out0.ap(), ag[3].ap(), tanhc.ap())
            vec.drain()
            vec.tensor_sub(tmp0.ap(), out0.ap(), pib[s].ap()[:, 160:192])
            vec.drain()
            vec.tensor_mul(tmp1.ap(), ag[4].ap(), tmp0.ap())
            vec.drain()
            vec.tensor_add(htile.ap(), tmp1.ap(), pib[s].ap()[:, 160:192])
            vec.drain()
            vec.tensor_mul(ybuf[s].ap(), htile.ap(),
                           pib[s].ap()[:, 192:224]).then_inc(YB, 1)
            vec.tensor_copy(send[par].ap(), htile.ap()).then_inc(SD, 1)
            # ---- gpsimd: broadcast h_{t+1} ----
            gp.wait_ge(SD, j * 4 + (s + 1))
            gp.remote_dma_broadcast(
                recv[(s + 1) % 4].ap()[:, bass.ts(pid_sv, 32)],
                send[par].ap(),
                remote_sem=RS[(s + 1) % 4], local_sem=LS[par],
                rdests=rdests,
            ).then_inc(PR, 1)
            gp.wait_ge(PR, j * 4 + (s + 2))
            gp.trigger_dma(1)
            # ---- sync: store y, prefetch pi t+4 ----
            syn.wait_ge(YB, j * 4 + (s + 1))
            syn.dma_start(
                ydram[:, bass.DynSlice(j * 4 + s, 1), :], ybuf[s].ap()
            ).then_inc(YS[s], 16)
            syn.dma_start(
                pib[s].ap(), pi[:, bass.DynSlice(j * 4 + (s + 4), 1), :, :]
            ).then_inc(PID[s], 16)

    nc.all_engine_barrier()

    # ============ phase 3: transpose y to [t, b, h] + int8 quantize ==========
    for s in range(4):
        syn.wait_ge(YS[s], 16 * NJ)     # all recurrence y stores landed
    for g in range(T // 4):
        u = g % 4
        if g >= 4:
            syn.wait_ge(TRD, g - 3)     # ytin[u] free: quantize g-4 done
        syn.dma_start(ytin[u].ap(),
                      ydram[:, 4 * g:4 * (g + 1), :]).then_inc(YLD, 16)
        vec.wait_ge(YLD, 16 * (g + 1))
        if g >= 4:
            vec.wait_ge(YSD, 80 * (g - 3))  # q8/rsc[u] free: stores g-4 done
        vec.transpose(ytr[u].ap(), ytin[u].ap())
        vec.drain()
        # per-partition absmax -> dequant scale rmax/127, quant mult 127/rmax
        vec.tensor_reduce(rmax.ap(), ytr[u].ap(), axis=mybir.AxisListType.X,
                          op=mybir.AluOpType.max, apply_absolute_value=True)
        vec.drain()
        vec.tensor_scalar_max(rinv.ap(), rmax.ap(), ceps.ap()[:, 0:1])
        vec.drain()
        vec.tensor_mul(rsc[u].ap(), rinv.ap(), c127.ap())
        vec.drain()
        vec.reciprocal(rinv.ap(), rsc[u].ap())
        vec.drain()
        vec.tensor_scalar_mul(q8[u].ap(), ytr[u].ap(),
                              rinv.ap()[:, 0:1]).then_inc(TRD, 1)
        syn.wait_ge(TRD, g + 1)
        for hb in range(4):
            syn.dma_start(
                yout[4 * g:4 * (g + 1), :, 32 * hb:32 * (hb + 1)]
                .rearrange("t b hh -> b t hh"),
                q8[u].ap()[32 * hb:32 * (hb + 1), :],
            ).then_inc(YSD, 16)
        syn.dma_start(yscd[g:g + 1, :], rsc[u].ap()).then_inc(YSD, 16)

    nc.all_engine_barrier()
    nc.compile()
    return nc


# ---------------------------------------------------------------------------
# Host side: cached jit over shard_map, minimal-byte transfers.
_EXEC = {}
_CONST = {}


def _get_exec(T):
    if T in _EXEC:
        return _EXEC[T]
    import jax
    from jax.sharding import Mesh, PartitionSpec, NamedSharding
    from jax.experimental.shard_map import shard_map
    from concourse import bass2jax, mybir as _mb
    import jax.numpy as jnp

    nc = build_program(T)
    bass2jax.install_neuronx_cc_hook()

    partition_name = (nc.partition_id_tensor.name
                      if nc.partition_id_tensor else None)
    in_names, out_names, out_avals = [], [], []
    for alloc in nc.m.functions[0].allocations:
        if not isinstance(alloc, _mb.MemoryLocationSet):
            continue
        name = alloc.memorylocations[0].name
        if alloc.kind == "ExternalInput":
            if name != partition_name:
                in_names.append(name)
        elif alloc.kind == "ExternalOutput":
            shape = tuple(alloc.tensor_shape)
            dtype = _mb.dt.np(alloc.dtype)
            out_names.append(name)
            out_avals.append(jax.core.ShapedArray(shape, dtype))
    n_params = len(in_names)
    n_outs = len(out_names)
    all_in_names = list(in_names) + list(out_names)
    if partition_name is not None:
        all_in_names.append(partition_name)

    def _body(*args):
        operands = list(args)
        if partition_name is not None:
            operands.append(bass2jax.partition_id_tensor())
        outs = bass2jax._bass_exec_p.bind(
            *operands,
            out_avals=tuple(out_avals),
            in_names=tuple(all_in_names),
            out_names=tuple(out_names),
            lowering_input_output_aliases=(),
            sim_require_finite=True,
            sim_require_nnan=True,
            nc=nc,
        )
        return tuple(outs)

    devices = jax.devices()[:NCORES]
    mesh = Mesh(np.asarray(devices), ("core",))
    in_specs = (PartitionSpec("core"),) * (n_params + n_outs)
    out_specs = (PartitionSpec("core"),) * n_outs
    donate = tuple(range(n_params, n_params + n_outs))
    sharded = jax.jit(shard_map(_body, mesh=mesh, in_specs=in_specs,
                                out_specs=out_specs, check_rep=False),
                      donate_argnums=donate, keep_unused=True)
    shard0 = NamedSharding(mesh, PartitionSpec("core"))

    def _zeros():
        return tuple(
            jnp.zeros((NCORES * a.shape[0], *a.shape[1:]), a.dtype)
            for a in out_avals)

    zeros_fn = jax.jit(_zeros, out_shardings=(shard0,) * n_outs)

    dev_order = {d.id: i for i, d in enumerate(devices)}
    _EXEC[T] = dict(nc=nc, sharded=sharded, zeros_fn=zeros_fn,
                    in_names=in_names, out_names=out_names,
                    dev_order=dev_order)
    return _EXEC[T]


def _make_globals(inputs, W_in, b_in, W_s, b_s, lengths, T):
    bf = ml_dtypes.bfloat16
    x = np.ascontiguousarray(np.asarray(inputs, np.float32)).astype(bf)

    W_in6 = np.asarray(W_in, np.float32).reshape(NPI, TPD, HC, D)
    w1t_g = np.zeros((NCORES * D, NPI * HC), bf)
    w1t_g[:TPD * D] = (W_in6.transpose(1, 3, 0, 2)
                       .reshape(TPD * D, NPI * HC).astype(bf))
    W_s5 = np.asarray(W_s, np.float32).reshape(NG, TPD, HC, H)
    w2t_g = np.zeros((NCORES * H, NG * HC), bf)
    w2t_g[:TPD * H] = (W_s5.transpose(1, 3, 0, 2)
                       .reshape(TPD * H, NG * HC).astype(bf))

    b1_g = np.zeros((NCORES * HC, NPI), np.float32)
    b1_g[:TPD * HC] = (np.asarray(b_in, np.float32)
                       .reshape(NPI, TPD, HC).transpose(1, 2, 0)
                       .reshape(TPD * HC, NPI))
    b2_g = np.zeros((NCORES * HC, NG), np.float32)
    b2_g[:TPD * HC] = (np.asarray(b_s, np.float32)
                       .reshape(NG, TPD, HC).transpose(1, 2, 0)
                       .reshape(TPD * HC, NG))

    if "ident" not in _CONST:
        _CONST["ident"] = np.ascontiguousarray(
            np.tile(np.eye(128, dtype=bf), (NCORES, 1)))
        _CONST["ones1"] = np.ones((NCORES, 128), bf)
    lengths = np.asarray(lengths).astype(np.int64)
    mask = (np.arange(T)[:, None] < lengths[None, :]).astype(bf)  # [T,B]
    mrow_g = np.ascontiguousarray(
        np.broadcast_to(mask.reshape(1, T * 32), (NCORES, T * 32)))

    return {"xsh": x, "w1t": w1t_g, "w2t": w2t_g, "b1": b1_g, "b2": b2_g,
            "ident": _CONST["ident"], "ones1": _CONST["ones1"],
            "mrow": mrow_g}


def kernel(inputs, W_in, b_in, W_s, b_s, lengths):
    from concurrent.futures import ThreadPoolExecutor

    T = np.asarray(inputs).shape[1]
    ex = _get_exec(T)
    g = _make_globals(inputs, W_in, b_in, W_s, b_s, lengths, T)
    zeros = ex["zeros_fn"]()
    out_arrs = ex["sharded"](*[g[n] for n in ex["in_names"]], *zeros)
    y_g = out_arrs[ex["out_names"].index("y")]
    s_g = out_arrs[ex["out_names"].index("yscale")]
    order = lambda arr: sorted(arr.addressable_shards,
                               key=lambda s: ex["dev_order"][s.device.id])
    yshards, sshards = order(y_g), order(s_g)
    out = np.empty((B, T, H), np.float32)
    G = T // 4

    def fetch(k):
        q = np.asarray(yshards[k].data)              # [T,32,128] int8
        sc = np.asarray(sshards[k].data)             # [G,128] f32
        # scale for (t,b,h) = sc[t//4, 32*(h//32) + b]
        qf = q.astype(np.float32).reshape(G, 4, 32, 4, 32)  # g,tl,b,hb,hh
        qf *= sc.reshape(G, 4, 32).transpose(0, 2, 1)[:, None, :, :, None]
        out[:, :, HC * k:HC * (k + 1)] = \
            qf.reshape(T, 32, 128).transpose(1, 0, 2)

    with ThreadPoolExecutor(TPD) as pool:
        list(pool.map(fetch, range(TPD)))
    return out


if __name__ == "__main__":
    print("kernel module; call kernel(**inputs)")
